# revision 15
# baseline (speedup 1.0000x reference)
"""Trainium2 Bass kernel for a dense transformer block with sigmoid attention.

Shapes (hardcoded): B=8, N=1024, C=768, H=12 heads, D=64, HID=3072.
Sharding: data-parallel over batch -- one batch element per NeuronCore (8 cores).

Math notes (host-side folding, all exact reassociations in fp32):
  - ln1 affine folded into qkv_w / qkv_b  (h = LN0(x); qkv = h @ (qkv_w*w1).T + b')
  - attention scale D**-0.5 folded into q columns of qkv_w (power of 2, exact)
  - ls1 folded into proj_w/proj_b;  ln2 affine folded into w1/b1;  ls2 into w2/b2
  - matmuls run in bf16 (fp32 PSUM accumulate); the residual stream stays fp32.
    Since both branches are scaled by layerscale ~1e-6, output error is ~1e-8.

On-chip layout per core:
  activations are kept "feature-major" (features on partitions, tokens on free
  dim) for all weight matmuls; layernorm runs token-major and the normalized
  activations are transposed to feature-major via DMA-transpose (bf16 XBAR).
"""

import os

import numpy as np
import ml_dtypes

B, N, C, H = 8, 1024, 768, 12
D = C // H           # 64
HID = 4 * C          # 3072
LN_EPS = 1e-5
P = 128
KC = C // P          # 6   C chunks
NT = N // P          # 8   token chunks
MQK = 2 * C // P     # 12  q+k feature chunks
MHID = HID // P      # 24  hidden chunks
NCORES = 8

BF16 = ml_dtypes.bfloat16

LAST_EXEC_TIME_NS = None
LAST_TRACE_PATH = None


def _build_program(attn_bias: float, has_vbias: bool, has_bproj: bool, has_b2: bool,
                   use_dma_transpose: bool = True):
    import concourse.bass as bass
    import concourse.mybir as mybir
    import concourse.tile as tile
    from concourse import bacc
    from concourse.masks import make_identity
    from contextlib import ExitStack

    dt = mybir.dt
    FP32 = dt.float32
    BF = dt.bfloat16
    AF = mybir.ActivationFunctionType
    OP = mybir.AluOpType

    nc = bacc.Bacc("TRN2", debug=False, enable_asserts=False,
                   target_bir_lowering=False, num_devices=NCORES)

    x_d = nc.dram_tensor("x", [N, C], FP32, kind="ExternalInput").ap()
    wqkv_d = nc.dram_tensor("wqkv_t", [C, 3 * C], BF, kind="ExternalInput").ap()
    bqkv_d = nc.dram_tensor("bqkv", [3 * C], FP32, kind="ExternalInput").ap()
    wproj_d = nc.dram_tensor("wproj_t", [C, C], BF, kind="ExternalInput").ap()
    bproj_d = nc.dram_tensor("bproj", [C], FP32, kind="ExternalInput").ap()
    w1_d = nc.dram_tensor("w1_t", [C, HID], BF, kind="ExternalInput").ap()
    b1_d = nc.dram_tensor("b1", [HID], FP32, kind="ExternalInput").ap()
    w2_d = nc.dram_tensor("w2_t", [HID, C], BF, kind="ExternalInput").ap()
    b2_d = nc.dram_tensor("b2", [C], FP32, kind="ExternalInput").ap()
    out_d = nc.dram_tensor("out", [N, C], FP32, kind="ExternalOutput").ap()

    def bcast_row(src_1d_ap, p=P):
        # [L] dram vector -> [p, L] partition-broadcast AP (step 0 on partitions)
        return bass.AP(tensor=src_1d_ap.tensor, offset=src_1d_ap.offset,
                       ap=[[0, p]] + list(src_1d_ap.ap))

    with ExitStack() as ctx:
        tc = ctx.enter_context(tile.TileContext(nc))

        consts = ctx.enter_context(tc.tile_pool(name="consts", bufs=1))
        stream = ctx.enter_context(tc.tile_pool(name="stream", bufs=2))
        stats_p = ctx.enter_context(tc.tile_pool(name="stats", bufs=4))
        # arena: one long-lived pool (bufs=1); pool size = sum of tag slot sizes,
        # so sequentially-dead tensors share a tag to reuse the slot:
        #   t48:  qkT(24) -> m1T(48)      t36a: wqkv(27.6) -> w1(36)
        #   t36b: sT x12 (16) -> w2(36)   t24:  hT(12) -> x2(24)
        #   t12a: oT -> h2T               t12b: v          t9: wproj
        arena = ctx.enter_context(tc.tile_pool(name="arena", bufs=1))

        # ---- constants / biases ----
        eps_sb = consts.tile([P, 1], FP32, tag="eps")
        nc.vector.memset(eps_sb, LN_EPS)
        ab_sb = consts.tile([P, 1], FP32, tag="attn_bias")
        nc.vector.memset(ab_sb, attn_bias)
        bqkv_sb = consts.tile([P, 3 * C // P], FP32, tag="bqkv")
        nc.sync.dma_start(out=bqkv_sb, in_=bqkv_d.rearrange("(t p) -> p t", p=P))
        b1_sb = consts.tile([P, MHID], FP32, tag="b1")
        nc.sync.dma_start(out=b1_sb, in_=b1_d.rearrange("(t p) -> p t", p=P))
        if has_vbias:
            vb_bc = consts.tile([P, C], FP32, tag="vb_bc")
            nc.gpsimd.dma_start(out=vb_bc, in_=bcast_row(bqkv_d[2 * C:]))
        if has_bproj:
            bproj_bc = consts.tile([P, C], FP32, tag="bproj_bc")
            nc.gpsimd.dma_start(out=bproj_bc, in_=bcast_row(bproj_d))
        if has_b2:
            b2_bc = consts.tile([P, C], FP32, tag="b2_bc")
            nc.gpsimd.dma_start(out=b2_bc, in_=bcast_row(b2_d))
        if not use_dma_transpose:
            ident = consts.tile([P, P], BF, tag="ident")
            make_identity(nc, ident)

        # ---- weights (per-chunk DMAs so consumers can start early) ----
        wqkv_sb = arena.tile([P, KC, 3 * C], BF, tag="t36a", name="wqkv_sb")
        for k in range(KC):
            nc.sync.dma_start(out=wqkv_sb[:, k, :], in_=wqkv_d[k * P:(k + 1) * P, :])
        wproj_sb = arena.tile([P, KC, C], BF, tag="t9", name="wproj_sb")
        for k in range(KC):
            nc.sync.dma_start(out=wproj_sb[:, k, :], in_=wproj_d[k * P:(k + 1) * P, :])

        # ---- layernorm (token-major) -> write transposed bf16 chunks ----
        def layernorm_to_T(i, src_ap, hT_tile, ps_pool):
            stats = stats_p.tile([P, 3, 6], FP32, tag="ln_stats")
            xg = src_ap.rearrange("p (g d) -> p g d", g=3)
            for g in range(3):
                nc.vector.bn_stats(out=stats[:, g, :], in_=xg[:, g, :])
            mv = stats_p.tile([P, 2], FP32, tag="ln_mv")
            nc.vector.bn_aggr(out=mv, in_=stats)
            std = stats_p.tile([P, 1], FP32, tag="ln_std")
            nc.scalar.activation(std, mv[:, 1:2], AF.Sqrt, bias=eps_sb)
            rstd = stats_p.tile([P, 1], FP32, tag="ln_rstd")
            nc.vector.reciprocal(rstd, std)
            ht = stream.tile([P, C], BF, tag="ln_ht")
            nc.vector.tensor_scalar(out=ht, in0=src_ap, scalar1=mv[:, 0:1],
                                    scalar2=rstd, op0=OP.subtract, op1=OP.mult)
            for j in range(KC):
                dst = hT_tile[:, j, i * P:(i + 1) * P]
                src = ht[:, j * P:(j + 1) * P]
                if use_dma_transpose:
                    nc.sync.dma_start(out=dst, in_=src, transpose=True)
                else:
                    pt = ps_pool.tile([P, P], FP32, tag="tr_psum")
                    nc.tensor.transpose(pt, src, ident)
                    nc.vector.tensor_copy(out=dst, in_=pt)

        # ================= Phase A: LN1 + h^T =================
        hT = arena.tile([P, KC, N], BF, tag="t24", name="hT")
        with tc.tile_pool(name="psA0", bufs=2, space="PSUM") as psA0:
            for i in range(NT):
                xt = stream.tile([P, C], FP32, tag="io_t", name="x_in")
                nc.sync.dma_start(out=xt, in_=x_d[i * P:(i + 1) * P, :])
                layernorm_to_T(i, xt, hT, psA0)

        # ================= Phase A2: qkv projections =================
        qkT = arena.tile([P, MQK, N], BF, tag="t48", name="qkT")
        v_sb = arena.tile([P, NT, C], BF, tag="t12b", name="v_sb")
        with tc.tile_pool(name="psA", bufs=4, space="PSUM") as psA:
            # q,k feature-major: qkT[o_chunk][o_part, n]
            for mc in range(MQK):
                for half in range(2):
                    ps = psA.tile([P, 512], FP32, tag="ps_qk")
                    for k in range(KC):
                        nc.tensor.matmul(ps,
                                         lhsT=wqkv_sb[:, k, mc * P:(mc + 1) * P],
                                         rhs=hT[:, k, half * 512:(half + 1) * 512],
                                         start=(k == 0), stop=(k == KC - 1))
                    nc.scalar.activation(out=qkT[:, mc, half * 512:(half + 1) * 512],
                                         in_=ps, func=AF.Identity,
                                         bias=bqkv_sb[:, mc:mc + 1])
            # v token-major: v_sb[tok_chunk][tok_part, o]
            for i in range(NT):
                for half, nw in ((0, 512), (1, 256)):
                    ps = psA.tile([P, 512], FP32, tag="ps_v")
                    for k in range(KC):
                        nc.tensor.matmul(ps[:, :nw],
                                         lhsT=hT[:, k, i * P:(i + 1) * P],
                                         rhs=wqkv_sb[:, k, 2 * C + half * 512:
                                                     2 * C + half * 512 + nw],
                                         start=(k == 0), stop=(k == KC - 1))
                    dst = v_sb[:, i, half * 512:half * 512 + nw]
                    if has_vbias:
                        nc.vector.tensor_add(out=dst, in0=ps[:, :nw],
                                             in1=vb_bc[:, half * 512:half * 512 + nw])
                    else:
                        nc.scalar.copy(out=dst, in_=ps[:, :nw])

        # ================= Phase B: sigmoid attention =================
        # w1 load here: reuses wqkv slot (t36a); DMA overlaps all of phase B
        w1_sb = arena.tile([P, KC, HID], BF, tag="t36a", name="w1_sb")
        for k in range(KC):
            nc.sync.dma_start(out=w1_sb[:, k, :], in_=w1_d[k * P:(k + 1) * P, :])

        oT = arena.tile([P, KC, N], BF, tag="t12a", name="oT")
        with tc.tile_pool(name="psS", bufs=3, space="PSUM") as psS, \
             tc.tile_pool(name="psO", bufs=4, space="PSUM") as psO:
            for h in range(H):
                po = (h % 2) * D          # partition offset within chunk
                cq = h // 2               # q chunk
                ck = KC + h // 2          # k chunk
                sT = arena.tile([P, NT, N], BF, tag="t36b", name=f"sT_{h}")
                # scores^T[m, n] = sum_d k^T[d, m] * q^T[d, n]   (scale in q)
                for mc in range(NT):
                    for half in range(2):
                        ps = psS.tile([P, 512], FP32, tag="ps_s")
                        nc.tensor.matmul(ps,
                                         lhsT=qkT[po:po + D, ck, mc * P:(mc + 1) * P],
                                         rhs=qkT[po:po + D, cq, half * 512:(half + 1) * 512],
                                         start=True, stop=True)
                        nc.scalar.activation(out=sT[:, mc, half * 512:(half + 1) * 512],
                                             in_=ps, func=AF.Sigmoid, bias=ab_sb)
                # o^T[d, n] = sum_m v[m, d] * s^T[m, n]
                pso = [psO.tile([D, 512], FP32, tag="ps_o", name=f"ps_o_{half}")
                       for half in range(2)]
                for mc in range(NT):
                    for half in range(2):
                        nc.tensor.matmul(pso[half],
                                         lhsT=v_sb[:, mc, h * D:(h + 1) * D],
                                         rhs=sT[:, mc, half * 512:(half + 1) * 512],
                                         start=(mc == 0), stop=(mc == NT - 1))
                for half in range(2):
                    nc.vector.tensor_copy(
                        out=oT[po:po + D, h // 2, half * 512:(half + 1) * 512],
                        in_=pso[half])

        # ================= Phase C: proj + residual (fp32) =================
        # w2 load here: reuses sT slot (t36b); DMA overlaps phases C / LN2 / mlp1
        w2_sb = arena.tile([P, MHID, C], BF, tag="t36b", name="w2_sb")
        for k in range(MHID):
            nc.sync.dma_start(out=w2_sb[:, k, :], in_=w2_d[k * P:(k + 1) * P, :])

        x2 = arena.tile([P, NT, C], FP32, tag="t24", name="x2")
        with tc.tile_pool(name="psC", bufs=3, space="PSUM") as psC:
            for i in range(NT):
                xt = stream.tile([P, C], FP32, tag="io_t", name="x_in")
                nc.sync.dma_start(out=xt, in_=x_d[i * P:(i + 1) * P, :])
                for half, nw in ((0, 512), (1, 256)):
                    ps = psC.tile([P, 512], FP32, tag="ps_c")
                    for k in range(KC):
                        nc.tensor.matmul(ps[:, :nw],
                                         lhsT=oT[:, k, i * P:(i + 1) * P],
                                         rhs=wproj_sb[:, k, half * 512:half * 512 + nw],
                                         start=(k == 0), stop=(k == KC - 1))
                    dst = x2[:, i, half * 512:half * 512 + nw]
                    nc.vector.tensor_add(out=dst, in0=ps[:, :nw],
                                         in1=xt[:, half * 512:half * 512 + nw])
                    if has_bproj:
                        nc.vector.tensor_add(out=dst, in0=dst,
                                             in1=bproj_bc[:, half * 512:half * 512 + nw])
        # ================= Phase D: LN2 + MLP + residual =================
        h2T = arena.tile([P, KC, N], BF, tag="t12a", name="h2T")
        with tc.tile_pool(name="psD0", bufs=2, space="PSUM") as psD0:
            for i in range(NT):
                layernorm_to_T(i, x2[:, i, :], h2T, psD0)

        m1T = arena.tile([P, MHID, N], BF, tag="t48", name="m1T")
        with tc.tile_pool(name="psD", bufs=4, space="PSUM") as psD:
            for mc in range(MHID):
                for half in range(2):
                    ps = psD.tile([P, 512], FP32, tag="ps_m1")
                    for k in range(KC):
                        nc.tensor.matmul(ps,
                                         lhsT=w1_sb[:, k, mc * P:(mc + 1) * P],
                                         rhs=h2T[:, k, half * 512:(half + 1) * 512],
                                         start=(k == 0), stop=(k == KC - 1))
                    nc.scalar.activation(out=m1T[:, mc, half * 512:(half + 1) * 512],
                                         in_=ps, func=AF.Gelu, bias=b1_sb[:, mc:mc + 1])

        with tc.tile_pool(name="psE", bufs=3, space="PSUM") as psE:
            for i in range(NT):
                ot = stream.tile([P, C], FP32, tag="io_t", name="out_t")
                for half, nw in ((0, 512), (1, 256)):
                    ps = psE.tile([P, 512], FP32, tag="ps_m2")
                    for k in range(MHID):
                        nc.tensor.matmul(ps[:, :nw],
                                         lhsT=m1T[:, k, i * P:(i + 1) * P],
                                         rhs=w2_sb[:, k, half * 512:half * 512 + nw],
                                         start=(k == 0), stop=(k == MHID - 1))
                    dst = ot[:, half * 512:half * 512 + nw]
                    nc.vector.tensor_add(out=dst, in0=ps[:, :nw],
                                         in1=x2[:, i, half * 512:half * 512 + nw])
                    if has_b2:
                        nc.vector.tensor_add(out=dst, in0=dst,
                                             in1=b2_bc[:, half * 512:half * 512 + nw])
                nc.sync.dma_start(out=out_d[i * P:(i + 1) * P, :], in_=ot)

    nc.finalize()  # Bacc: runs register allocation + codegen passes
    return nc


def kernel(x, ln1_w, ln1_b, qkv_w, qkv_b, proj_w, proj_b, attn_bias,
           ls1, ln2_w, ln2_b, w1, b1, w2, b2, ls2):
    global LAST_EXEC_TIME_NS, LAST_TRACE_PATH
    from concourse.bass_utils import run_bass_kernel_spmd

    x = np.asarray(x, np.float32)
    f32 = lambda a: np.asarray(a, np.float32)
    ln1_w, ln1_b, qkv_w, qkv_b = f32(ln1_w), f32(ln1_b), f32(qkv_w), f32(qkv_b)
    proj_w, proj_b, ls1 = f32(proj_w), f32(proj_b), f32(ls1)
    ln2_w, ln2_b, w1, b1, w2, b2, ls2 = (f32(ln2_w), f32(ln2_b), f32(w1),
                                         f32(b1), f32(w2), f32(b2), f32(ls2))
    ab = float(np.asarray(attn_bias, np.float32))

    # ---- host-side weight folding (fp32, then cast to bf16) ----
    scale = D ** -0.5
    qkv_w_eff = qkv_w * ln1_w[None, :]
    bqkv_eff = qkv_b + qkv_w @ ln1_b
    wqkv_t = np.ascontiguousarray(qkv_w_eff.T)
    wqkv_t[:, :C] *= scale
    bqkv_eff = bqkv_eff.copy()
    bqkv_eff[:C] *= scale
    wproj_t = np.ascontiguousarray((proj_w * ls1[:, None]).T)
    bproj_eff = proj_b * ls1
    w1_t = np.ascontiguousarray((w1 * ln2_w[None, :]).T)
    b1_eff = b1 + w1 @ ln2_b
    w2_t = np.ascontiguousarray((w2 * ls2[:, None]).T)
    b2_eff = b2 * ls2

    has_vbias = bool(np.any(bqkv_eff[2 * C:] != 0.0))
    has_bproj = bool(np.any(bproj_eff != 0.0))
    has_b2 = bool(np.any(b2_eff != 0.0))

    nc = _build_program(ab, has_vbias, has_bproj, has_b2,
                        use_dma_transpose=os.environ.get("KERNEL_PE_TRANSPOSE", "0") != "1")

    shared = {
        "wqkv_t": wqkv_t.astype(BF16),
        "bqkv": bqkv_eff.astype(np.float32),
        "wproj_t": wproj_t.astype(BF16),
        "bproj": bproj_eff.astype(np.float32),
        "w1_t": w1_t.astype(BF16),
        "b1": b1_eff.astype(np.float32),
        "w2_t": w2_t.astype(BF16),
        "b2": b2_eff.astype(np.float32),
    }
    in_maps = [dict(shared, x=np.ascontiguousarray(x[c])) for c in range(NCORES)]

    trace = os.environ.get("KERNEL_TRACE", "0") == "1"
    res = run_bass_kernel_spmd(nc, in_maps, core_ids=list(range(NCORES)),
                               trace=trace)
    LAST_EXEC_TIME_NS = res.exec_time_ns
    if res.instructions_and_trace is not None:
        LAST_TRACE_PATH = res.instructions_and_trace[1]
    return np.stack([r["out"] for r in res.results]).astype(np.float32)


if __name__ == "__main__":
    xs = np.random.default_rng(0).standard_normal((B, N, C), dtype=np.float32)
    outs = kernel(
        x=xs,
        ln1_w=np.ones(C, np.float32), ln1_b=np.zeros(C, np.float32),
        qkv_w=np.random.default_rng(1).uniform(-0.036, 0.036, (3 * C, C)).astype(np.float32),
        qkv_b=np.zeros(3 * C, np.float32),
        proj_w=np.random.default_rng(2).uniform(-0.036, 0.036, (C, C)).astype(np.float32),
        proj_b=np.zeros(C, np.float32),
        attn_bias=np.float32(-6.93),
        ls1=np.full(C, 1e-6, np.float32),
        ln2_w=np.ones(C, np.float32), ln2_b=np.zeros(C, np.float32),
        w1=np.random.default_rng(3).uniform(-0.036, 0.036, (HID, C)).astype(np.float32),
        b1=np.zeros(HID, np.float32),
        w2=np.random.default_rng(4).uniform(-0.018, 0.018, (C, HID)).astype(np.float32),
        b2=np.zeros(C, np.float32),
        ls2=np.full(C, 1e-6, np.float32),
    )
    print("ok", outs.shape, float(np.abs(outs).max()))
